# revision 23
# baseline (speedup 1.0000x reference)
"""CLIP encoder layer (B=4, S=2048, H=768, NH=12, FF=3072) on 8 trn2 cores.

Sharding: data-parallel over batch (4) x query-halves (2) = 8 cores, no
cross-core collectives. Each core receives its batch's tokens permuted so
its 1024 queries come first (attention is permutation-invariant over keys),
computes K/V over the full 2048-token sequence, and emits the final layer
output for its 1024 tokens.

v2: restructured for Tensor-engine p-state residency. The PE only reaches
2.4 GHz after ~3us of gapless execution, so the kernel is organized as two
query chunks of 512 whose attention windows are padded with independent PE
work: chunk A's softmax shadow absorbs the V projection + chunk B's Q
projection, and chunk B's softmax shadow absorbs chunk A's entire MLP.
Scores for two heads share one [128,1024] EXP; softmax normalization uses
reciprocal_approx_fast on the denominator row + an f32r broadcast matmul.
All layernorm/residual tensors are bf16 in SBUF; matmuls are bf16 with
fp32 PSUM.
"""

import os
import numpy as np
import ml_dtypes

import concourse.bass as bass
import concourse.mybir as mybir
import concourse.tile as tile
from concourse.bass_utils import run_bass_kernel_spmd
F32 = mybir.dt.float32
F32R = mybir.dt.float32r
BF16 = mybir.dt.bfloat16
AF = mybir.ActivationFunctionType
ALU = mybir.AluOpType

B, S, H, NH, HD, FF = 4, 2048, 768, 12, 64, 3072
EPS = 1e-5
QL = 1024          # queries per core
NT = S // 128      # 16 key token tiles (full seq)
NTQ = QL // 128    # 8 local query token tiles
FC = H // 128      # 6 feature chunks
OC1 = FF // 128    # 24 fc1 output chunks
HW = HD + 1        # augmented head width (ones column for the denominator)
CQ = 512           # query chunk (2 chunks per core)
NCH = QL // CQ
MASK_NEG = -30.0

LAST = {}  # exec stats from the most recent run


def _split_multi_waits(nc):
    """This walrus build accepts at most ONE sync wait per instruction;
    hoist extra waits onto same-engine NoOps inserted just before."""
    n = 0
    for f in nc.m.functions:
        for blk in f.blocks:
            insts = list(blk.instructions)
            if not any(
                i.sync_info and len(i.sync_info.on_wait) > 1 for i in insts
            ):
                continue
            new = []
            for inst in insts:
                si = inst.sync_info
                if si is not None and len(si.on_wait) > 1:
                    waits = list(si.on_wait)
                    for w in waits[:-1]:
                        nop = mybir.InstNoOp(
                            name=f"I-{nc.next_id()}", text_hint="split_wait"
                        )
                        nop.engine = inst.engine
                        nop.sync_info = mybir.SyncInfo(on_wait=[w], on_update=[])
                        new.append(nop)
                        n += 1
                    inst.sync_info = mybir.SyncInfo(
                        on_wait=[waits[-1]], on_update=list(si.on_update)
                    )
                new.append(inst)
            del blk.instructions[:]
            for i in new:
                blk.add_instruction(i)
    return n


def _build_program():
    nc = bass.Bass("TRN2", target_bir_lowering=False, debug=False, num_devices=8)

    d_x = nc.dram_tensor("x", [S, H], F32, kind="ExternalInput").ap()
    d_maskb = nc.dram_tensor("maskb", [128, NT], F32, kind="ExternalInput").ap()
    d_wq = nc.dram_tensor("wq", [H, H], BF16, kind="ExternalInput").ap()
    d_wk = nc.dram_tensor("wk", [H, H], BF16, kind="ExternalInput").ap()
    d_wv = nc.dram_tensor("wv", [H, H], BF16, kind="ExternalInput").ap()
    d_wo = nc.dram_tensor("wo", [H, H], BF16, kind="ExternalInput").ap()
    d_w1 = nc.dram_tensor("w1", [OC1, 128, H], BF16, kind="ExternalInput").ap()
    d_w2 = nc.dram_tensor("w2", [FC, 128, FF], BF16, kind="ExternalInput").ap()
    d_bq = nc.dram_tensor("bq", [128, FC], F32, kind="ExternalInput").ap()
    d_bk = nc.dram_tensor("bk", [128, FC], F32, kind="ExternalInput").ap()
    d_bo = nc.dram_tensor("bo", [128, FC], F32, kind="ExternalInput").ap()
    d_b2 = nc.dram_tensor("b2", [128, FC], F32, kind="ExternalInput").ap()
    d_b1 = nc.dram_tensor("b1", [128, OC1], F32, kind="ExternalInput").ap()
    d_bvb = nc.dram_tensor("bvb", [128, H], F32, kind="ExternalInput").ap()
    d_id16 = nc.dram_tensor("id16", [128, 128], BF16, kind="ExternalInput").ap()
    d_id32 = nc.dram_tensor("id32", [128, 128], F32, kind="ExternalInput").ap()
    d_ones = nc.dram_tensor("ones", [1, 128], F32, kind="ExternalInput").ap()
    d_out = nc.dram_tensor("out", [QL, H], F32, kind="ExternalOutput").ap()

    from contextlib import ExitStack
    with tile.TileContext(nc) as tc:
        with ExitStack() as stack:
            ep = lambda *a, **kw: stack.enter_context(tc.tile_pool(*a, **kw))
            cpool = ep(name="const", bufs=1)
            pool_st = ep(name="stat", bufs=4)
            # --- PSUM: exactly 8 banks ---------------------------------
            pool_sc = ep(name="psc", bufs=2, space="PSUM")   # 4 banks
            pool_pv = ep(name="ppv", bufs=2, space="PSUM")   # 2 banks
            pool_mm = ep(name="pmm", bufs=2, space="PSUM")   # 2 banks
            # --- big persistent SBUF tensors ---------------------------
            pool_k = ep(name="kfm", bufs=1)
            pool_v = ep(name="vaug", bufs=1)
            pool_q = ep(name="qfm", bufs=1)
            pool_wo = ep(name="wop", bufs=1)
            pool_p = ep(name="probs", bufs=3)
            pool_ctx = ep(name="ctx", bufs=2)
            pool_rb = ep(name="rb", bufs=3)
            # ---- constants (id16 first: the LN1 transposes need it) ----
            id16 = cpool.tile([128, 128], BF16, tag="id16", name="id16")
            nc.sync.dma_start(id16[:], d_id16)
            id32 = cpool.tile([128, 128], F32, tag="id32", name="id32")
            nc.sync.dma_start(id32[:], d_id32)
            maskb = cpool.tile([128, NT], F32, tag="maskb", name="maskb")
            nc.sync.dma_start(maskb[:], d_maskb)
            bq_sb = cpool.tile([128, FC], F32, tag="bq", name="bq")
            nc.sync.dma_start(bq_sb[:], d_bq)
            bk_sb = cpool.tile([128, FC], F32, tag="bk", name="bk")
            nc.sync.dma_start(bk_sb[:], d_bk)
            bo_sb = cpool.tile([128, FC], F32, tag="bo", name="bo")
            nc.sync.dma_start(bo_sb[:], d_bo)
            b2_sb = cpool.tile([128, FC], F32, tag="b2", name="b2")
            nc.sync.dma_start(b2_sb[:], d_b2)
            b1_sb = cpool.tile([128, OC1], F32, tag="b1", name="b1")
            nc.sync.dma_start(b1_sb[:], d_b1)
            bvb_sb = cpool.tile([128, H], F32, tag="bvb", name="bvb")
            nc.sync.dma_start(bvb_sb[:], d_bvb)
            ones_sb = cpool.tile([1, 128], F32, tag="ones", name="ones")
            nc.sync.dma_start(ones_sb[:], d_ones)
            ones16 = cpool.tile([HW, 128], BF16, tag="ones16", name="ones16")
            nc.vector.memset(ones16[:], 1.0)
            eps_t = cpool.tile([128, 1], F32, tag="eps", name="eps")
            nc.vector.memset(eps_t[:], EPS)

            def layer_norm_tile(xt, out_bf16):
                """token-major LN (no gain/bias — folded into weights)."""
                bn = pool_st.tile([128, 2, 6], F32, tag="bn", name="bn")
                nc.vector.bn_stats(bn[:, 0, :], xt[:, 0:384])
                nc.vector.bn_stats(bn[:, 1, :], xt[:, 384:768])
                st = pool_st.tile([128, 2], F32, tag="st", name="st")
                nc.vector.bn_aggr(st[:], bn[:])
                negmu = pool_st.tile([128, 1], F32, tag="negmu", name="negmu")
                nc.vector.tensor_scalar_mul(negmu[:], st[:, 0:1], -1.0)
                sq = pool_st.tile([128, 1], F32, tag="sq", name="sq")
                nc.scalar.activation(sq[:], st[:, 1:2], AF.Sqrt, bias=eps_t[:])
                rstd = pool_st.tile([128, 1], F32, tag="rstd", name="rstd")
                nc.vector.reciprocal_approx_fast(rstd[:], sq[:])
                nc.vector.tensor_scalar(
                    out_bf16, xt[:], negmu[:], rstd[:],
                    op0=ALU.add, op1=ALU.mult,
                )

            def transpose_6(src_bf16, dst, dst_view):
                """Transpose six [128,128] blocks of a token-major bf16 tile
                into feature-major positions of dst. dst_view(lo, n) must
                return a [128, n, 128] AP over feature blocks lo..lo+n."""
                pt1 = pool_mm.tile([128, 512], BF16, tag="pmm", name="pt1")
                for fc in range(4):
                    nc.tensor.transpose(
                        pt1[:, fc * 128:(fc + 1) * 128],
                        src_bf16[:, fc * 128:(fc + 1) * 128], id16[:],
                    )
                nc.scalar.activation(
                    dst_view(0, 4), pt1[:].rearrange("p (f c) -> p f c", c=128),
                    AF.Copy,
                )
                pt2 = pool_mm.tile([128, 512], BF16, tag="pmm", name="pt2")
                for fc in range(4, 6):
                    nc.tensor.transpose(
                        pt2[:, (fc - 4) * 128:(fc - 3) * 128],
                        src_bf16[:, fc * 128:(fc + 1) * 128], id16[:],
                    )
                nc.scalar.activation(
                    dst_view(4, 2),
                    pt2[:, 0:256].rearrange("p (f c) -> p f c", c=128),
                    AF.Copy,
                )

            # ================================================================
            # Phase 1: load x, LN1, transpose to feature-major XNF
            # ================================================================
            with (
                tc.tile_pool(name="xnf", bufs=1) as pool_xnf,
                tc.tile_pool(name="wqkv", bufs=1) as pool_wqkv,
            ):
                XNF = pool_xnf.tile([128, FC * S], BF16, tag="xnf", name="xnf")

                with (
                    tc.tile_pool(name="x_in", bufs=3) as pool_xin,
                    tc.tile_pool(name="xn1", bufs=3) as pool_xn1,
                ):
                    for t in range(NT):
                        xt = pool_xin.tile([128, H], F32, tag="xin", name="xin")
                        nc.sync.dma_start(xt[:], d_x[t * 128:(t + 1) * 128, :])
                        xn = pool_xn1.tile([128, H], BF16, tag="xn1", name="xn1")
                        layer_norm_tile(xt, xn[:])

                        def xnf_view(lo, n, t=t):
                            return XNF[:].rearrange(
                                "p (f c) -> p f c", f=FC
                            )[:, lo:lo + n, t * 128:(t + 1) * 128]

                        transpose_6(xn, XNF, xnf_view)

                # ---- weights for QKV/out-proj ------------------------------
                wq_sb = pool_wqkv.tile([128, FC * H], BF16, tag="wq", name="wq")
                wk_sb = pool_wqkv.tile([128, FC * H], BF16, tag="wk", name="wk")
                wv_sb = pool_wqkv.tile([128, FC * H], BF16, tag="wv", name="wv")
                wo_sb = pool_wo.tile([128, FC * H], BF16, tag="wo", name="wo")
                for kc in range(FC):
                    nc.sync.dma_start(
                        wq_sb[:, kc * H:(kc + 1) * H],
                        d_wq[kc * 128:(kc + 1) * 128, :])
                    nc.sync.dma_start(
                        wk_sb[:, kc * H:(kc + 1) * H],
                        d_wk[kc * 128:(kc + 1) * 128, :])
                    nc.sync.dma_start(
                        wv_sb[:, kc * H:(kc + 1) * H],
                        d_wv[kc * 128:(kc + 1) * 128, :])
                    nc.sync.dma_start(
                        wo_sb[:, kc * H:(kc + 1) * H],
                        d_wo[kc * 128:(kc + 1) * 128, :])

                KF = pool_k.tile([128, FC * S], BF16, tag="kfm", name="kfm")
                VA = pool_v.tile([128, NT * NH * HW], BF16, tag="vaug", name="vaug")
                QF = pool_q.tile([128, FC * QL], BF16, tag="qfm", name="qfm")

                # ================================================================
                # Phase 2: K projection (all), V projection (first half),
                #          Q projection (chunk A)
                # ================================================================
                def k_proj(oc, g):
                    ps = pool_mm.tile([128, 512], F32, tag="pmm", name="kp")
                    for kc in range(FC):
                        nc.tensor.matmul(
                            ps[:],
                            wk_sb[:, kc * H + oc * 128:kc * H + (oc + 1) * 128],
                            XNF[:, kc * S + g * 512:kc * S + (g + 1) * 512],
                            start=(kc == 0), stop=(kc == FC - 1),
                        )
                    nc.vector.tensor_scalar(
                        KF[:, oc * S + g * 512:oc * S + (g + 1) * 512], ps[:],
                        bk_sb[:, oc:oc + 1], None, op0=ALU.add,
                    )

                def v_proj(t):
                    nc.vector.memset(
                        VA[:, t * NH * HW:(t + 1) * NH * HW].rearrange(
                            "p (h w) -> p h w", w=HW
                        )[:, :, HD],
                        1.0,
                    )
                    for v0, vn in ((0, 512), (512, 256)):
                        ps = pool_mm.tile([128, 512], F32, tag="pmm", name="vp")
                        for kc in range(FC):
                            nc.tensor.matmul(
                                ps[:, 0:vn],
                                XNF[:, kc * S + t * 128:kc * S + (t + 1) * 128],
                                wv_sb[:, kc * H + v0:kc * H + v0 + vn],
                                start=(kc == 0), stop=(kc == FC - 1),
                            )
                        nh = vn // HD
                        nc.vector.tensor_tensor(
                            VA[:, t * NH * HW:(t + 1) * NH * HW].rearrange(
                                "p (h w) -> p h w", w=HW
                            )[:, v0 // HD:v0 // HD + nh, 0:HD],
                            ps[:, 0:vn].rearrange("p (h w) -> p h w", w=HD),
                            bvb_sb[:, v0:v0 + vn].rearrange(
                                "p (h w) -> p h w", w=HD
                            ),
                            op=ALU.add,
                        )

                def q_proj(oc, c):
                    ps = pool_mm.tile([128, 512], F32, tag="pmm", name="qp")
                    for kc in range(FC):
                        nc.tensor.matmul(
                            ps[:],
                            wq_sb[:, kc * H + oc * 128:kc * H + (oc + 1) * 128],
                            XNF[:, kc * S + c * CQ:kc * S + (c + 1) * CQ],
                            start=(kc == 0), stop=(kc == FC - 1),
                        )
                    nc.vector.tensor_scalar(
                        QF[:, oc * QL + c * CQ:oc * QL + (c + 1) * CQ], ps[:],
                        bq_sb[:, oc:oc + 1], None, op0=ALU.add,
                    )

                for oc in range(FC):
                    for g in range(S // 512):
                        k_proj(oc, g)
                for t in range(NT // 2):
                    v_proj(t)
                for oc in range(FC):
                    q_proj(oc, 0)

                # ================================================================
                # Phase 3: attention chunk A (+ remaining V proj as PE filler)
                # ================================================================
                CTX = [
                    pool_ctx.tile([128, FC * CQ], BF16, tag="ctx", name="ctx")
                    for _ in range(NCH)
                ]

                def attn_chunk(c, fillers):
                    """Attention for queries [c*CQ, (c+1)*CQ). fillers: list of
                    (at_step, fn) emitted inside the hp=0 t-loop."""
                    fill = dict(fillers)
                    step = 0
                    for hp in range(NH // 2):
                        h0, h1 = 2 * hp, 2 * hp + 1
                        psc = {}
                        for h in (h0, h1):
                            psc[h] = pool_pv.tile(
                                [HW, CQ], F32, tag="ppv", name="ppv"
                            )
                        for t in range(NT):
                            sc = pool_sc.tile([128, 2 * CQ], F32, tag="psc", name="sc")
                            for i, h in enumerate((h0, h1)):
                                hof = (h % 2) * HD
                                nc.tensor.matmul(
                                    sc[:, i * CQ:(i + 1) * CQ],
                                    KF[hof:hof + HD,
                                       hp * S + t * 128:hp * S + (t + 1) * 128],
                                    QF[hof:hof + HD,
                                       hp * QL + c * CQ:hp * QL + (c + 1) * CQ],
                                    start=True, stop=True,
                                )
                            probs = pool_p.tile(
                                [128, 2 * CQ], BF16, tag="probs", name="probs"
                            )
                            nc.scalar.activation(
                                probs[:], sc[:], AF.Exp, bias=maskb[:, t:t + 1]
                            )
                            for i, h in enumerate((h0, h1)):
                                nc.tensor.matmul(
                                    psc[h][0:HW, :],
                                    VA[:, t * NH * HW + h * HW:
                                       t * NH * HW + (h + 1) * HW],
                                    probs[:, i * CQ:(i + 1) * CQ],
                                    start=(t == 0), stop=(t == NT - 1),
                                )
                            if step in fill:
                                fill.pop(step)()
                            step += 1
                        for h in (h0, h1):
                            hof = (h % 2) * HD
                            # One DVE pass frees the PV accumulator; the
                            # normalization pipeline then runs off-PSUM.
                            craw = pool_rb.tile([HW, CQ], BF16, tag="craw", name="craw")
                            nc.vector.tensor_copy(craw[:], psc[h][0:HW, :])
                            pb = pool_mm.tile([128, 512], F32, tag="pmm", name="pb")
                            nc.tensor.matmul(
                                pb[0:HD, 0:CQ],
                                ones16[HD:HW, 0:HD],
                                craw[HD:HW, :],
                                start=True, stop=True,
                            )
                            rbb = pool_rb.tile([HD, CQ], F32, tag="rbb", name="rbb")
                            nc.vector.reciprocal_approx_fast(
                                rbb[:], pb[0:HD, 0:CQ]
                            )
                            nc.vector.tensor_tensor(
                                CTX[c][hof:hof + HD,
                                       hp * CQ:(hp + 1) * CQ],
                                craw[0:HD, :], rbb[:], op=ALU.mult,
                            )

                fillers_a = [
                    (i, (lambda t=t: v_proj(t))) for i, t in enumerate(range(NT // 2, NT))
                ]
                fillers_a += [
                    (8 + i, (lambda oc=oc: q_proj(oc, 1))) for i, oc in enumerate(range(FC))
                ]
                attn_chunk(0, fillers_a)
            # pool_xnf + pool_wqkv close here: XNF/wq/wk/wv space freed for MLP

            # ================================================================
            # Phase 4+: per-chunk out-proj + LN2 + MLP; chunk A's MLP overlaps
            # chunk B's attention via the tile scheduler.
            # ================================================================
            mstack = stack.enter_context(ExitStack())
            mp = lambda *a, **kw: mstack.enter_context(tc.tile_pool(*a, **kw))
            pool_x2 = mp(name="x2", bufs=NTQ)
            pool_xn2 = mp(name="xn2", bufs=2)
            pool_h = mp(name="hfm", bufs=1)
            pool_y = mp(name="yo", bufs=6)
            pool_o = mp(name="of", bufs=6)
            pool_xr = mp(name="xres", bufs=2)
            pool_out = mp(name="outk", bufs=2)
            pool_w1 = mp(name="w1s", bufs=4)
            pool_w2 = mp(name="w2s", bufs=2)
            pool_xn2t = mp(name="xn2t", bufs=2)

            X2_tiles = [None] * NTQ
            XN2 = [
                pool_xn2.tile([128, FC * CQ], BF16, tag="xn2", name="xn2")
                for _ in range(NCH)
            ]
            HF = pool_h.tile([128, OC1 * CQ], BF16, tag="hfm", name="hfm")

            def out_proj_ln2(c):
                Y = []
                for oc in range(FC):
                    ps = pool_mm.tile([128, 512], F32, tag="pmm", name="op")
                    for kc in range(FC):
                        nc.tensor.matmul(
                            ps[:],
                            wo_sb[:, kc * H + oc * 128:kc * H + (oc + 1) * 128],
                            CTX[c][:, kc * CQ:(kc + 1) * CQ],
                            start=(kc == 0), stop=(kc == FC - 1),
                        )
                    yt = pool_y.tile([128, CQ], F32, tag="yo", name="yo")
                    nc.vector.tensor_scalar(
                        yt[:], ps[:], bo_sb[:, oc:oc + 1], None, op0=ALU.add,
                    )
                    Y.append(yt)
                for tc_ in range(CQ // 128):
                    t = c * (CQ // 128) + tc_
                    xr = pool_xr.tile([128, H], F32, tag="xres", name="xres")
                    nc.sync.dma_start(xr[:], d_x[t * 128:(t + 1) * 128, :])
                    x2t = pool_x2.tile([128, H], BF16, tag="x2", name="x2")
                    pt1 = pool_mm.tile([128, 512], F32, tag="pmm", name="ytr1")
                    for oc in range(4):
                        nc.tensor.transpose(
                            pt1[:, oc * 128:(oc + 1) * 128],
                            Y[oc][:, tc_ * 128:(tc_ + 1) * 128], id32[:],
                        )
                    nc.vector.tensor_tensor(
                        x2t[:, 0:512], pt1[:], xr[:, 0:512], op=ALU.add
                    )
                    pt2 = pool_mm.tile([128, 512], F32, tag="pmm", name="ytr2")
                    for oc in range(4, 6):
                        nc.tensor.transpose(
                            pt2[:, (oc - 4) * 128:(oc - 3) * 128],
                            Y[oc][:, tc_ * 128:(tc_ + 1) * 128], id32[:],
                        )
                    nc.vector.tensor_tensor(
                        x2t[:, 512:768], pt2[:, 0:256], xr[:, 512:768],
                        op=ALU.add,
                    )
                    X2_tiles[t] = x2t
                    xn = pool_xn2t.tile([128, H], BF16, tag="xn2t", name="xn2t")
                    layer_norm_tile(x2t, xn[:])

                    def xn2_view(lo, n, c=c, tc_=tc_):
                        return XN2[c][:].rearrange(
                            "p (f q) -> p f q", f=FC
                        )[:, lo:lo + n, tc_ * 128:(tc_ + 1) * 128]

                    transpose_6(xn, XN2[c], xn2_view)

            def mlp_chunk(c):
                for oc in range(OC1):
                    w1t = pool_w1.tile([128, H], BF16, tag="w1s", name="w1s")
                    nc.sync.dma_start(w1t[:], d_w1[oc])
                    ps = pool_mm.tile([128, 512], F32, tag="pmm", name="f1")
                    for kc in range(FC):
                        nc.tensor.matmul(
                            ps[:],
                            w1t[:, kc * 128:(kc + 1) * 128],
                            XN2[c][:, kc * CQ:(kc + 1) * CQ],
                            start=(kc == 0), stop=(kc == FC - 1),
                        )
                    nc.scalar.activation(
                        HF[:, oc * CQ:(oc + 1) * CQ], ps[:], AF.Gelu,
                        bias=b1_sb[:, oc:oc + 1],
                    )
                O = []
                for oc in range(FC):
                    w2t = pool_w2.tile([128, FF], BF16, tag="w2s", name="w2s")
                    nc.sync.dma_start(w2t[:], d_w2[oc])
                    ps = pool_mm.tile([128, 512], F32, tag="pmm", name="f2")
                    for kc in range(OC1):
                        nc.tensor.matmul(
                            ps[:],
                            w2t[:, kc * 128:(kc + 1) * 128],
                            HF[:, kc * CQ:(kc + 1) * CQ],
                            start=(kc == 0), stop=(kc == OC1 - 1),
                        )
                    ot = pool_o.tile([128, CQ], F32, tag="of", name="of")
                    nc.vector.tensor_scalar(
                        ot[:], ps[:], b2_sb[:, oc:oc + 1], None, op0=ALU.add,
                    )
                    O.append(ot)
                for tc_ in range(CQ // 128):
                    t = c * (CQ // 128) + tc_
                    outt = pool_out.tile([128, H], F32, tag="outk", name="outk")
                    pt1 = pool_mm.tile([128, 512], F32, tag="pmm", name="otr1")
                    for oc in range(4):
                        nc.tensor.transpose(
                            pt1[:, oc * 128:(oc + 1) * 128],
                            O[oc][:, tc_ * 128:(tc_ + 1) * 128], id32[:],
                        )
                    nc.vector.tensor_tensor(
                        outt[:, 0:512], pt1[:], X2_tiles[t][:, 0:512],
                        op=ALU.add,
                    )
                    pt2 = pool_mm.tile([128, 512], F32, tag="pmm", name="otr2")
                    for oc in range(4, 6):
                        nc.tensor.transpose(
                            pt2[:, (oc - 4) * 128:(oc - 3) * 128],
                            O[oc][:, tc_ * 128:(tc_ + 1) * 128], id32[:],
                        )
                    nc.vector.tensor_tensor(
                        outt[:, 512:768], pt2[:, 0:256],
                        X2_tiles[t][:, 512:768], op=ALU.add,
                    )
                    nc.sync.dma_start(
                        d_out[t * 128:(t + 1) * 128, :], outt[:]
                    )

            out_proj_ln2(0)
            mlp_chunk(0)
            attn_chunk(1, [])
            out_proj_ln2(1)
            mlp_chunk(1)

    return nc


_NC_CACHE = None
_SPLIT_DONE = False


def _get_program(split=False):
    global _NC_CACHE, _SPLIT_DONE
    if _NC_CACHE is None:
        _NC_CACHE = _build_program()
        # Populate .instr bytes for extended InstISA subclasses (the
        # custom-DVE reciprocal) — raw Bass skips this Bacc pass.
        mybir.codegen_inst_isa_subclasses(_NC_CACHE)
    if split and not _SPLIT_DONE:
        _split_multi_waits(_NC_CACHE)
        _SPLIT_DONE = True
    return _NC_CACHE


def _prep_inputs(inputs):
    f32 = np.float32
    bf = ml_dtypes.bfloat16
    hs = np.asarray(inputs["hidden_states"], f32)
    am = np.asarray(inputs["attention_mask"])
    ln1_g = np.asarray(inputs["ln1_g"], f32)
    ln1_b = np.asarray(inputs["ln1_b"], f32)
    ln2_g = np.asarray(inputs["ln2_g"], f32)
    ln2_b = np.asarray(inputs["ln2_b"], f32)
    wq, bq = np.asarray(inputs["wq"], f32), np.asarray(inputs["bq"], f32)
    wk, bk = np.asarray(inputs["wk"], f32), np.asarray(inputs["bk"], f32)
    wv, bv = np.asarray(inputs["wv"], f32), np.asarray(inputs["bv"], f32)
    wo, bo = np.asarray(inputs["wo"], f32), np.asarray(inputs["bo"], f32)
    w1, b1 = np.asarray(inputs["w1"], f32), np.asarray(inputs["b1"], f32)
    w2, b2 = np.asarray(inputs["w2"], f32), np.asarray(inputs["b2"], f32)

    # Fold LN1 gain + 1/sqrt(hd) into wq/wk/wv rows, LN1 bias into the proj
    # biases; LN2 gain/bias into w1/b1 likewise.
    scale = HD ** -0.5
    wq_f = np.ascontiguousarray((ln1_g[:, None] * wq * scale).astype(bf))
    wk_f = np.ascontiguousarray((ln1_g[:, None] * wk).astype(bf))
    wv_f = np.ascontiguousarray((ln1_g[:, None] * wv).astype(bf))
    wo_f = np.ascontiguousarray(wo.astype(bf))
    bq_eff = (bq + ln1_b @ wq) * scale
    bk_eff = bk + ln1_b @ wk
    bv_eff = bv + ln1_b @ wv
    w1_f = np.ascontiguousarray(
        (ln2_g[:, None] * w1).reshape(FC, 128, OC1, 128)
        .transpose(2, 1, 0, 3).reshape(OC1, 128, H).astype(bf)
    )
    b1_eff = b1 + ln2_b @ w1
    w2_f = np.ascontiguousarray(
        w2.reshape(OC1, 128, FC, 128).transpose(2, 1, 0, 3)
        .reshape(FC, 128, FF).astype(bf)
    )

    def col_layout(v, n):
        return np.ascontiguousarray(v.reshape(n, 128).T.astype(f32))

    common = {
        "wq": wq_f, "wk": wk_f, "wv": wv_f, "wo": wo_f,
        "w1": w1_f, "w2": w2_f,
        "bq": col_layout(bq_eff, FC), "bk": col_layout(bk_eff, FC),
        "bo": col_layout(bo, FC), "b2": col_layout(b2, FC),
        "b1": col_layout(b1_eff, OC1),
        "bvb": np.ascontiguousarray(
            np.broadcast_to(bv_eff[None, :], (128, H)).astype(f32)
        ),
        "id16": np.eye(128).astype(bf),
        "id32": np.eye(128, dtype=f32),
        "ones": np.ones((1, 128), np.float32),
    }

    in_maps = []
    for c in range(8):
        b, half = divmod(c, 2)
        idx = np.r_[half * QL:(half + 1) * QL, (1 - half) * QL:(2 - half) * QL]
        xp = np.ascontiguousarray(hs[b][idx])
        mb = np.where(am[b][idx] != 0, 0.0, MASK_NEG).astype(np.float32)
        m = dict(common)
        m["x"] = xp
        m["maskb"] = np.ascontiguousarray(mb.reshape(NT, 128).T)
        in_maps.append(m)
    return in_maps


def kernel(**inputs):
    in_maps = _prep_inputs(inputs)
    nc = _get_program(split=True)
    trace = os.environ.get("BASS_KERNEL_TRACE", "") == "1"
    if trace:
        _register_ntff_hook()
    res = run_bass_kernel_spmd(nc, in_maps, list(range(8)), trace=trace)
    LAST["exec_time_ns"] = res.exec_time_ns
    LAST["mean_exec_time_ns"] = res.mean_exec_time_ns
    LAST["res"] = res

    out = np.empty((B, S, H), np.float32)
    for c in range(8):
        b, half = divmod(c, 2)
        out[b, half * QL:(half + 1) * QL] = res.results[c]["out"]
    return out


def _register_ntff_hook():
    """The agent image's antenv lacks axon_hooks; reconstruct it so
    run_bass_kernel_spmd(trace=True) can capture NTFF profiles."""
    import sys, types
    if "antenv.axon_hooks" in sys.modules:
        return
    try:
        import antenv
        from trn_agent_boot.trn_boot import _ntff_profile_via_ctypes
        mod = types.ModuleType("antenv.axon_hooks")
        hook = _ntff_profile_via_ctypes("/opt/axon/libaxon_pjrt.so")
        mod.get_axon_ntff_profile_hook = lambda: hook
        mod.set_axon_ntff_profile_hook = lambda h: None
        sys.modules["antenv.axon_hooks"] = mod
        antenv.axon_hooks = mod
    except Exception:
        pass


# revision 24
# speedup vs baseline: 1.2078x; 1.2078x over previous
"""CLIP encoder layer (B=4, S=2048, H=768, NH=12, FF=3072) on 8 trn2 cores.

Sharding: data-parallel over batch (4) x query-halves (2) = 8 cores, no
cross-core collectives. Each core receives its batch's tokens permuted so
its 1024 queries come first (attention is permutation-invariant over keys),
computes K/V over the full 2048-token sequence, and emits the final layer
output for its 1024 tokens.

v2: restructured for Tensor-engine p-state residency. The PE only reaches
2.4 GHz after ~3us of gapless execution, so the kernel is organized as two
query chunks of 512 whose attention windows are padded with independent PE
work: chunk A's softmax shadow absorbs the V projection + chunk B's Q
projection, and chunk B's softmax shadow absorbs chunk A's entire MLP.
Scores for two heads share one [128,1024] EXP; softmax normalization uses
reciprocal_approx_fast on the denominator row + an f32r broadcast matmul.
All layernorm/residual tensors are bf16 in SBUF; matmuls are bf16 with
fp32 PSUM.
"""

import os
import numpy as np
import ml_dtypes

import concourse.bass as bass
import concourse.mybir as mybir
import concourse.tile as tile
from concourse.bass_utils import run_bass_kernel_spmd
F32 = mybir.dt.float32
F32R = mybir.dt.float32r
BF16 = mybir.dt.bfloat16
AF = mybir.ActivationFunctionType
ALU = mybir.AluOpType

B, S, H, NH, HD, FF = 4, 2048, 768, 12, 64, 3072
EPS = 1e-5
QL = 1024          # queries per core
NT = S // 128      # 16 key token tiles (full seq)
NTQ = QL // 128    # 8 local query token tiles
FC = H // 128      # 6 feature chunks
OC1 = FF // 128    # 24 fc1 output chunks
HW = HD + 1        # augmented head width (ones column for the denominator)
CQ = 512           # query chunk (2 chunks per core)
NCH = QL // CQ
MASK_NEG = -30.0

LAST = {}  # exec stats from the most recent run


def _split_multi_waits(nc):
    """This walrus build accepts at most ONE sync wait per instruction;
    hoist extra waits onto same-engine NoOps inserted just before."""
    n = 0
    for f in nc.m.functions:
        for blk in f.blocks:
            insts = list(blk.instructions)
            if not any(
                i.sync_info and len(i.sync_info.on_wait) > 1 for i in insts
            ):
                continue
            new = []
            for inst in insts:
                si = inst.sync_info
                if si is not None and len(si.on_wait) > 1:
                    waits = list(si.on_wait)
                    for w in waits[:-1]:
                        nop = mybir.InstNoOp(
                            name=f"I-{nc.next_id()}", text_hint="split_wait"
                        )
                        nop.engine = inst.engine
                        nop.sync_info = mybir.SyncInfo(on_wait=[w], on_update=[])
                        new.append(nop)
                        n += 1
                    inst.sync_info = mybir.SyncInfo(
                        on_wait=[waits[-1]], on_update=list(si.on_update)
                    )
                new.append(inst)
            del blk.instructions[:]
            for i in new:
                blk.add_instruction(i)
    return n


def _build_program():
    nc = bass.Bass("TRN2", target_bir_lowering=False, debug=False, num_devices=8)

    d_x = nc.dram_tensor("x", [S, H], F32, kind="ExternalInput").ap()
    d_maskb = nc.dram_tensor("maskb", [128, NT], F32, kind="ExternalInput").ap()
    d_wq = nc.dram_tensor("wq", [H, H], BF16, kind="ExternalInput").ap()
    d_wk = nc.dram_tensor("wk", [H, H], BF16, kind="ExternalInput").ap()
    d_wv = nc.dram_tensor("wv", [H, H], BF16, kind="ExternalInput").ap()
    d_wo = nc.dram_tensor("wo", [H, H], BF16, kind="ExternalInput").ap()
    d_w1 = nc.dram_tensor("w1", [OC1, 128, H], BF16, kind="ExternalInput").ap()
    d_w2 = nc.dram_tensor("w2", [FC, 128, FF], BF16, kind="ExternalInput").ap()
    d_bq = nc.dram_tensor("bq", [128, FC], F32, kind="ExternalInput").ap()
    d_bk = nc.dram_tensor("bk", [128, FC], F32, kind="ExternalInput").ap()
    d_bo = nc.dram_tensor("bo", [128, FC], F32, kind="ExternalInput").ap()
    d_b2 = nc.dram_tensor("b2", [128, FC], F32, kind="ExternalInput").ap()
    d_b1 = nc.dram_tensor("b1", [128, OC1], F32, kind="ExternalInput").ap()
    d_bvb = nc.dram_tensor("bvb", [128, H], F32, kind="ExternalInput").ap()
    d_id16 = nc.dram_tensor("id16", [128, 128], BF16, kind="ExternalInput").ap()
    d_id32 = nc.dram_tensor("id32", [128, 128], F32, kind="ExternalInput").ap()
    d_ones = nc.dram_tensor("ones", [1, 128], F32, kind="ExternalInput").ap()
    d_out = nc.dram_tensor("out", [QL, H], F32, kind="ExternalOutput").ap()

    from contextlib import ExitStack
    with tile.TileContext(nc) as tc:
        with ExitStack() as stack:
            ep = lambda *a, **kw: stack.enter_context(tc.tile_pool(*a, **kw))
            cpool = ep(name="const", bufs=1)
            pool_st = ep(name="stat", bufs=4)
            # --- PSUM: exactly 8 banks ---------------------------------
            pool_sc = ep(name="psc", bufs=2, space="PSUM")   # 4 banks
            pool_pv = ep(name="ppv", bufs=2, space="PSUM")   # 2 banks
            pool_mm = ep(name="pmm", bufs=2, space="PSUM")   # 2 banks
            # --- big persistent SBUF tensors ---------------------------
            pool_k = ep(name="kfm", bufs=1)
            pool_v = ep(name="vaug", bufs=1)
            pool_q = ep(name="qfm", bufs=1)
            pool_wo = ep(name="wop", bufs=1)
            pool_p = ep(name="probs", bufs=3)
            pool_ctx = ep(name="ctx", bufs=2)
            pool_rb = ep(name="rb", bufs=3)
            # ---- constants (id16 first: the LN1 transposes need it) ----
            id16 = cpool.tile([128, 128], BF16, tag="id16", name="id16")
            nc.sync.dma_start(id16[:], d_id16)
            id32 = cpool.tile([128, 128], F32, tag="id32", name="id32")
            nc.sync.dma_start(id32[:], d_id32)
            maskb = cpool.tile([128, NT], F32, tag="maskb", name="maskb")
            nc.sync.dma_start(maskb[:], d_maskb)
            bq_sb = cpool.tile([128, FC], F32, tag="bq", name="bq")
            nc.sync.dma_start(bq_sb[:], d_bq)
            bk_sb = cpool.tile([128, FC], F32, tag="bk", name="bk")
            nc.sync.dma_start(bk_sb[:], d_bk)
            bo_sb = cpool.tile([128, FC], F32, tag="bo", name="bo")
            nc.sync.dma_start(bo_sb[:], d_bo)
            b2_sb = cpool.tile([128, FC], F32, tag="b2", name="b2")
            nc.sync.dma_start(b2_sb[:], d_b2)
            b1_sb = cpool.tile([128, OC1], F32, tag="b1", name="b1")
            nc.sync.dma_start(b1_sb[:], d_b1)
            bvb_sb = cpool.tile([128, H], F32, tag="bvb", name="bvb")
            nc.sync.dma_start(bvb_sb[:], d_bvb)
            ones_sb = cpool.tile([1, 128], F32, tag="ones", name="ones")
            nc.sync.dma_start(ones_sb[:], d_ones)
            ones16 = cpool.tile([HW, 128], BF16, tag="ones16", name="ones16")
            nc.vector.memset(ones16[:], 1.0)
            eps_t = cpool.tile([128, 1], F32, tag="eps", name="eps")
            nc.vector.memset(eps_t[:], EPS)

            def layer_norm_tile(xt, out_bf16):
                """token-major LN (no gain/bias — folded into weights)."""
                bn = pool_st.tile([128, 2, 6], F32, tag="bn", name="bn")
                nc.vector.bn_stats(bn[:, 0, :], xt[:, 0:384])
                nc.vector.bn_stats(bn[:, 1, :], xt[:, 384:768])
                st = pool_st.tile([128, 2], F32, tag="st", name="st")
                nc.vector.bn_aggr(st[:], bn[:])
                negmu = pool_st.tile([128, 1], F32, tag="negmu", name="negmu")
                nc.vector.tensor_scalar_mul(negmu[:], st[:, 0:1], -1.0)
                sq = pool_st.tile([128, 1], F32, tag="sq", name="sq")
                nc.scalar.activation(sq[:], st[:, 1:2], AF.Sqrt, bias=eps_t[:])
                rstd = pool_st.tile([128, 1], F32, tag="rstd", name="rstd")
                nc.vector.reciprocal_approx_fast(rstd[:], sq[:])
                nc.vector.tensor_scalar(
                    out_bf16, xt[:], negmu[:], rstd[:],
                    op0=ALU.add, op1=ALU.mult,
                )

            def transpose_6(src_bf16, dst, dst_view):
                """Transpose six [128,128] blocks of a token-major bf16 tile
                into feature-major positions of dst. dst_view(lo, n) must
                return a [128, n, 128] AP over feature blocks lo..lo+n."""
                pt1 = pool_mm.tile([128, 512], BF16, tag="pmm", name="pt1")
                for fc in range(4):
                    nc.tensor.transpose(
                        pt1[:, fc * 128:(fc + 1) * 128],
                        src_bf16[:, fc * 128:(fc + 1) * 128], id16[:],
                    )
                nc.scalar.activation(
                    dst_view(0, 4), pt1[:].rearrange("p (f c) -> p f c", c=128),
                    AF.Copy,
                )
                pt2 = pool_mm.tile([128, 512], BF16, tag="pmm", name="pt2")
                for fc in range(4, 6):
                    nc.tensor.transpose(
                        pt2[:, (fc - 4) * 128:(fc - 3) * 128],
                        src_bf16[:, fc * 128:(fc + 1) * 128], id16[:],
                    )
                nc.scalar.activation(
                    dst_view(4, 2),
                    pt2[:, 0:256].rearrange("p (f c) -> p f c", c=128),
                    AF.Copy,
                )

            # ================================================================
            # Phase 1: load x, LN1, transpose to feature-major XNF
            # ================================================================
            with (
                tc.tile_pool(name="xnf", bufs=1) as pool_xnf,
                tc.tile_pool(name="wqkv", bufs=1) as pool_wqkv,
            ):
                XNF = pool_xnf.tile([128, FC * S], BF16, tag="xnf", name="xnf")

                with (
                    tc.tile_pool(name="x_in", bufs=3) as pool_xin,
                    tc.tile_pool(name="xn1", bufs=3) as pool_xn1,
                ):
                    for t in range(NT):
                        xt = pool_xin.tile([128, H], F32, tag="xin", name="xin")
                        nc.sync.dma_start(xt[:], d_x[t * 128:(t + 1) * 128, :])
                        xn = pool_xn1.tile([128, H], BF16, tag="xn1", name="xn1")
                        layer_norm_tile(xt, xn[:])

                        def xnf_view(lo, n, t=t):
                            return XNF[:].rearrange(
                                "p (f c) -> p f c", f=FC
                            )[:, lo:lo + n, t * 128:(t + 1) * 128]

                        transpose_6(xn, XNF, xnf_view)

                # ---- weights for QKV/out-proj ------------------------------
                wq_sb = pool_wqkv.tile([128, FC * H], BF16, tag="wq", name="wq")
                wk_sb = pool_wqkv.tile([128, FC * H], BF16, tag="wk", name="wk")
                wv_sb = pool_wqkv.tile([128, FC * H], BF16, tag="wv", name="wv")
                wo_sb = pool_wo.tile([128, FC * H], BF16, tag="wo", name="wo")
                for kc in range(FC):
                    nc.sync.dma_start(
                        wq_sb[:, kc * H:(kc + 1) * H],
                        d_wq[kc * 128:(kc + 1) * 128, :])
                    nc.sync.dma_start(
                        wk_sb[:, kc * H:(kc + 1) * H],
                        d_wk[kc * 128:(kc + 1) * 128, :])
                    nc.sync.dma_start(
                        wv_sb[:, kc * H:(kc + 1) * H],
                        d_wv[kc * 128:(kc + 1) * 128, :])
                    nc.sync.dma_start(
                        wo_sb[:, kc * H:(kc + 1) * H],
                        d_wo[kc * 128:(kc + 1) * 128, :])

                KF = pool_k.tile([128, FC * S], BF16, tag="kfm", name="kfm")
                VA = pool_v.tile([128, NT * NH * HW], BF16, tag="vaug", name="vaug")
                QF = pool_q.tile([128, FC * QL], BF16, tag="qfm", name="qfm")

                # ================================================================
                # Phase 2: K projection (all), V projection (first half),
                #          Q projection (chunk A)
                # ================================================================
                def k_proj(oc, g):
                    ps = pool_mm.tile([128, 512], F32, tag="pmm", name="kp")
                    for kc in range(FC):
                        nc.tensor.matmul(
                            ps[:],
                            wk_sb[:, kc * H + oc * 128:kc * H + (oc + 1) * 128],
                            XNF[:, kc * S + g * 512:kc * S + (g + 1) * 512],
                            start=(kc == 0), stop=(kc == FC - 1),
                        )
                    nc.vector.tensor_scalar(
                        KF[:, oc * S + g * 512:oc * S + (g + 1) * 512], ps[:],
                        bk_sb[:, oc:oc + 1], None, op0=ALU.add,
                    )

                def v_proj(t):
                    nc.vector.memset(
                        VA[:, t * NH * HW:(t + 1) * NH * HW].rearrange(
                            "p (h w) -> p h w", w=HW
                        )[:, :, HD],
                        1.0,
                    )
                    for v0, vn in ((0, 512), (512, 256)):
                        ps = pool_mm.tile([128, 512], F32, tag="pmm", name="vp")
                        for kc in range(FC):
                            nc.tensor.matmul(
                                ps[:, 0:vn],
                                XNF[:, kc * S + t * 128:kc * S + (t + 1) * 128],
                                wv_sb[:, kc * H + v0:kc * H + v0 + vn],
                                start=(kc == 0), stop=(kc == FC - 1),
                            )
                        nh = vn // HD
                        nc.vector.tensor_tensor(
                            VA[:, t * NH * HW:(t + 1) * NH * HW].rearrange(
                                "p (h w) -> p h w", w=HW
                            )[:, v0 // HD:v0 // HD + nh, 0:HD],
                            ps[:, 0:vn].rearrange("p (h w) -> p h w", w=HD),
                            bvb_sb[:, v0:v0 + vn].rearrange(
                                "p (h w) -> p h w", w=HD
                            ),
                            op=ALU.add,
                        )

                def q_proj(oc, c):
                    ps = pool_mm.tile([128, 512], F32, tag="pmm", name="qp")
                    for kc in range(FC):
                        nc.tensor.matmul(
                            ps[:],
                            wq_sb[:, kc * H + oc * 128:kc * H + (oc + 1) * 128],
                            XNF[:, kc * S + c * CQ:kc * S + (c + 1) * CQ],
                            start=(kc == 0), stop=(kc == FC - 1),
                        )
                    nc.vector.tensor_scalar(
                        QF[:, oc * QL + c * CQ:oc * QL + (c + 1) * CQ], ps[:],
                        bq_sb[:, oc:oc + 1], None, op0=ALU.add,
                    )

                for oc in range(FC):
                    for g in range(S // 512):
                        k_proj(oc, g)
                for t in range(NT // 2):
                    v_proj(t)
                for oc in range(FC):
                    q_proj(oc, 0)

                # ================================================================
                # Phase 3: attention chunk A (+ remaining V proj as PE filler)
                # ================================================================
                CTX = [
                    pool_ctx.tile([128, FC * CQ], BF16, tag="ctx", name="ctx")
                    for _ in range(NCH)
                ]

                def attn_chunk(c, fillers):
                    """Attention for queries [c*CQ, (c+1)*CQ). fillers: list of
                    (at_step, fn) emitted inside the hp=0 t-loop."""
                    fill = dict(fillers)
                    step = 0
                    for hp in range(NH // 2):
                        h0, h1 = 2 * hp, 2 * hp + 1
                        psc = {}
                        for h in (h0, h1):
                            psc[h] = pool_pv.tile(
                                [HW, CQ], F32, tag="ppv", name="ppv"
                            )
                        for t in range(NT):
                            sc = pool_sc.tile([128, 2 * CQ], F32, tag="psc", name="sc")
                            for i, h in enumerate((h0, h1)):
                                hof = (h % 2) * HD
                                nc.tensor.matmul(
                                    sc[:, i * CQ:(i + 1) * CQ],
                                    KF[hof:hof + HD,
                                       hp * S + t * 128:hp * S + (t + 1) * 128],
                                    QF[hof:hof + HD,
                                       hp * QL + c * CQ:hp * QL + (c + 1) * CQ],
                                    start=True, stop=True,
                                )
                            probs = pool_p.tile(
                                [128, 2 * CQ], BF16, tag="probs", name="probs"
                            )
                            nc.scalar.activation(
                                probs[:], sc[:], AF.Exp, bias=maskb[:, t:t + 1]
                            )
                            for i, h in enumerate((h0, h1)):
                                nc.tensor.matmul(
                                    psc[h][0:HW, :],
                                    VA[:, t * NH * HW + h * HW:
                                       t * NH * HW + (h + 1) * HW],
                                    probs[:, i * CQ:(i + 1) * CQ],
                                    start=(t == 0), stop=(t == NT - 1),
                                )
                            if step in fill:
                                fill.pop(step)()
                            step += 1
                        for h in (h0, h1):
                            hof = (h % 2) * HD
                            # One DVE pass frees the PV accumulator; the
                            # normalization pipeline then runs off-PSUM.
                            craw = pool_rb.tile([HW, CQ], BF16, tag="craw", name="craw")
                            nc.vector.tensor_copy(craw[:], psc[h][0:HW, :])
                            pb = pool_mm.tile([128, 512], F32, tag="pmm", name="pb")
                            nc.tensor.matmul(
                                pb[0:HD, 0:CQ],
                                ones16[HD:HW, 0:HD],
                                craw[HD:HW, :],
                                start=True, stop=True,
                            )
                            rbb = pool_rb.tile([HD, CQ], F32, tag="rbb", name="rbb")
                            nc.vector.reciprocal_approx_fast(
                                rbb[:], pb[0:HD, 0:CQ]
                            )
                            nc.vector.tensor_tensor(
                                CTX[c][hof:hof + HD,
                                       hp * CQ:(hp + 1) * CQ],
                                craw[0:HD, :], rbb[:], op=ALU.mult,
                            )

                fillers_a = [
                    (i, (lambda t=t: v_proj(t))) for i, t in enumerate(range(NT // 2, NT))
                ]
                fillers_a += [
                    (8 + i, (lambda oc=oc: q_proj(oc, 1))) for i, oc in enumerate(range(FC))
                ]
                attn_chunk(0, fillers_a)
            # pool_xnf + pool_wqkv close here: XNF/wq/wk/wv space freed for MLP

            # ================================================================
            # Phase 4+: per-chunk out-proj + LN2 + MLP; chunk A's MLP overlaps
            # chunk B's attention via the tile scheduler.
            # ================================================================
            mstack = stack.enter_context(ExitStack())
            mp = lambda *a, **kw: mstack.enter_context(tc.tile_pool(*a, **kw))
            pool_x2 = mp(name="x2", bufs=NTQ)
            pool_xn2 = mp(name="xn2", bufs=2)
            pool_h = mp(name="hfm", bufs=1)
            pool_y = mp(name="yo", bufs=6)
            pool_o = mp(name="of", bufs=6)
            pool_xr = mp(name="xres", bufs=2)
            pool_out = mp(name="outk", bufs=2)
            pool_w1 = mp(name="w1s", bufs=4)
            pool_w2 = mp(name="w2s", bufs=2)
            pool_xn2t = mp(name="xn2t", bufs=2)

            X2_tiles = [None] * NTQ
            XN2 = [
                pool_xn2.tile([128, FC * CQ], BF16, tag="xn2", name="xn2")
                for _ in range(NCH)
            ]
            HF = pool_h.tile([128, OC1 * CQ], BF16, tag="hfm", name="hfm")

            def out_proj_ln2(c):
                NTC = CQ // 128
                Y = []
                for oc in range(FC):
                    ps = pool_mm.tile([128, 512], F32, tag="pmm", name="op")
                    for kc in range(FC):
                        nc.tensor.matmul(
                            ps[:],
                            wo_sb[:, kc * H + oc * 128:kc * H + (oc + 1) * 128],
                            CTX[c][:, kc * CQ:(kc + 1) * CQ],
                            start=(kc == 0), stop=(kc == FC - 1),
                        )
                    yt = pool_y.tile([128, CQ], F32, tag="yo", name="yo")
                    nc.vector.tensor_scalar(
                        yt[:], ps[:], bo_sb[:, oc:oc + 1], None, op0=ALU.add,
                    )
                    Y.append(yt)
                # LN2 stats are batched per chunk so the Sqrt runs once —
                # scattered Sqrts between EXPs thrash the ACT table (~1.5us
                # per swap).
                st4 = pool_st.tile([128, NTC, 2], F32, tag="st4", name="st4")
                for tc_ in range(NTC):
                    t = c * NTC + tc_
                    xr = pool_xr.tile([128, H], F32, tag="xres", name="xres")
                    nc.sync.dma_start(xr[:], d_x[t * 128:(t + 1) * 128, :])
                    x2t = pool_x2.tile([128, H], BF16, tag="x2", name="x2")
                    pt1 = pool_mm.tile([128, 512], F32, tag="pmm", name="ytr1")
                    for oc in range(4):
                        nc.tensor.transpose(
                            pt1[:, oc * 128:(oc + 1) * 128],
                            Y[oc][:, tc_ * 128:(tc_ + 1) * 128], id32[:],
                        )
                    nc.vector.tensor_tensor(
                        x2t[:, 0:512], pt1[:], xr[:, 0:512], op=ALU.add
                    )
                    pt2 = pool_mm.tile([128, 512], F32, tag="pmm", name="ytr2")
                    for oc in range(4, 6):
                        nc.tensor.transpose(
                            pt2[:, (oc - 4) * 128:(oc - 3) * 128],
                            Y[oc][:, tc_ * 128:(tc_ + 1) * 128], id32[:],
                        )
                    nc.vector.tensor_tensor(
                        x2t[:, 512:768], pt2[:, 0:256], xr[:, 512:768],
                        op=ALU.add,
                    )
                    X2_tiles[t] = x2t
                    bn = pool_st.tile([128, 2, 6], F32, tag="bn", name="bn")
                    nc.vector.bn_stats(bn[:, 0, :], x2t[:, 0:384])
                    nc.vector.bn_stats(bn[:, 1, :], x2t[:, 384:768])
                    nc.vector.bn_aggr(st4[:, tc_, :], bn[:])
                negmu4 = pool_st.tile([128, NTC], F32, tag="negmu4", name="negmu4")
                nc.vector.tensor_scalar_mul(negmu4[:], st4[:, :, 0], -1.0)
                sq4 = pool_st.tile([128, NTC], F32, tag="sq4", name="sq4")
                nc.scalar.activation(sq4[:], st4[:, :, 1], AF.Sqrt, bias=eps_t[:])
                rstd4 = pool_st.tile([128, NTC], F32, tag="rstd4", name="rstd4")
                nc.vector.reciprocal_approx_fast(rstd4[:], sq4[:])
                for tc_ in range(NTC):
                    t = c * NTC + tc_
                    xn = pool_xn2t.tile([128, H], BF16, tag="xn2t", name="xn2t")
                    nc.vector.tensor_scalar(
                        xn[:], X2_tiles[t][:], negmu4[:, tc_:tc_ + 1],
                        rstd4[:, tc_:tc_ + 1], op0=ALU.add, op1=ALU.mult,
                    )

                    def xn2_view(lo, n, c=c, tc_=tc_):
                        return XN2[c][:].rearrange(
                            "p (f q) -> p f q", f=FC
                        )[:, lo:lo + n, tc_ * 128:(tc_ + 1) * 128]

                    transpose_6(xn, XN2[c], xn2_view)

            def mlp_chunk(c):
                for oc in range(OC1):
                    w1t = pool_w1.tile([128, H], BF16, tag="w1s", name="w1s")
                    nc.sync.dma_start(w1t[:], d_w1[oc])
                    ps = pool_mm.tile([128, 512], F32, tag="pmm", name="f1")
                    for kc in range(FC):
                        nc.tensor.matmul(
                            ps[:],
                            w1t[:, kc * 128:(kc + 1) * 128],
                            XN2[c][:, kc * CQ:(kc + 1) * CQ],
                            start=(kc == 0), stop=(kc == FC - 1),
                        )
                    nc.scalar.activation(
                        HF[:, oc * CQ:(oc + 1) * CQ], ps[:], AF.Gelu,
                        bias=b1_sb[:, oc:oc + 1],
                    )
                O = []
                for oc in range(FC):
                    w2t = pool_w2.tile([128, FF], BF16, tag="w2s", name="w2s")
                    nc.sync.dma_start(w2t[:], d_w2[oc])
                    ps = pool_mm.tile([128, 512], F32, tag="pmm", name="f2")
                    for kc in range(OC1):
                        nc.tensor.matmul(
                            ps[:],
                            w2t[:, kc * 128:(kc + 1) * 128],
                            HF[:, kc * CQ:(kc + 1) * CQ],
                            start=(kc == 0), stop=(kc == OC1 - 1),
                        )
                    ot = pool_o.tile([128, CQ], F32, tag="of", name="of")
                    nc.vector.tensor_scalar(
                        ot[:], ps[:], b2_sb[:, oc:oc + 1], None, op0=ALU.add,
                    )
                    O.append(ot)
                for tc_ in range(CQ // 128):
                    t = c * (CQ // 128) + tc_
                    outt = pool_out.tile([128, H], F32, tag="outk", name="outk")
                    pt1 = pool_mm.tile([128, 512], F32, tag="pmm", name="otr1")
                    for oc in range(4):
                        nc.tensor.transpose(
                            pt1[:, oc * 128:(oc + 1) * 128],
                            O[oc][:, tc_ * 128:(tc_ + 1) * 128], id32[:],
                        )
                    nc.vector.tensor_tensor(
                        outt[:, 0:512], pt1[:], X2_tiles[t][:, 0:512],
                        op=ALU.add,
                    )
                    pt2 = pool_mm.tile([128, 512], F32, tag="pmm", name="otr2")
                    for oc in range(4, 6):
                        nc.tensor.transpose(
                            pt2[:, (oc - 4) * 128:(oc - 3) * 128],
                            O[oc][:, tc_ * 128:(tc_ + 1) * 128], id32[:],
                        )
                    nc.vector.tensor_tensor(
                        outt[:, 512:768], pt2[:, 0:256],
                        X2_tiles[t][:, 512:768], op=ALU.add,
                    )
                    nc.sync.dma_start(
                        d_out[t * 128:(t + 1) * 128, :], outt[:]
                    )

            out_proj_ln2(0)
            mlp_chunk(0)
            attn_chunk(1, [])
            out_proj_ln2(1)
            mlp_chunk(1)

    return nc


_NC_CACHE = None
_SPLIT_DONE = False


def _get_program(split=False):
    global _NC_CACHE, _SPLIT_DONE
    if _NC_CACHE is None:
        _NC_CACHE = _build_program()
        # Populate .instr bytes for extended InstISA subclasses (the
        # custom-DVE reciprocal) — raw Bass skips this Bacc pass.
        mybir.codegen_inst_isa_subclasses(_NC_CACHE)
    if split and not _SPLIT_DONE:
        _split_multi_waits(_NC_CACHE)
        _SPLIT_DONE = True
    return _NC_CACHE


def _prep_inputs(inputs):
    f32 = np.float32
    bf = ml_dtypes.bfloat16
    hs = np.asarray(inputs["hidden_states"], f32)
    am = np.asarray(inputs["attention_mask"])
    ln1_g = np.asarray(inputs["ln1_g"], f32)
    ln1_b = np.asarray(inputs["ln1_b"], f32)
    ln2_g = np.asarray(inputs["ln2_g"], f32)
    ln2_b = np.asarray(inputs["ln2_b"], f32)
    wq, bq = np.asarray(inputs["wq"], f32), np.asarray(inputs["bq"], f32)
    wk, bk = np.asarray(inputs["wk"], f32), np.asarray(inputs["bk"], f32)
    wv, bv = np.asarray(inputs["wv"], f32), np.asarray(inputs["bv"], f32)
    wo, bo = np.asarray(inputs["wo"], f32), np.asarray(inputs["bo"], f32)
    w1, b1 = np.asarray(inputs["w1"], f32), np.asarray(inputs["b1"], f32)
    w2, b2 = np.asarray(inputs["w2"], f32), np.asarray(inputs["b2"], f32)

    # Fold LN1 gain + 1/sqrt(hd) into wq/wk/wv rows, LN1 bias into the proj
    # biases; LN2 gain/bias into w1/b1 likewise.
    scale = HD ** -0.5
    wq_f = np.ascontiguousarray((ln1_g[:, None] * wq * scale).astype(bf))
    wk_f = np.ascontiguousarray((ln1_g[:, None] * wk).astype(bf))
    wv_f = np.ascontiguousarray((ln1_g[:, None] * wv).astype(bf))
    wo_f = np.ascontiguousarray(wo.astype(bf))
    bq_eff = (bq + ln1_b @ wq) * scale
    bk_eff = bk + ln1_b @ wk
    bv_eff = bv + ln1_b @ wv
    w1_f = np.ascontiguousarray(
        (ln2_g[:, None] * w1).reshape(FC, 128, OC1, 128)
        .transpose(2, 1, 0, 3).reshape(OC1, 128, H).astype(bf)
    )
    b1_eff = b1 + ln2_b @ w1
    w2_f = np.ascontiguousarray(
        w2.reshape(OC1, 128, FC, 128).transpose(2, 1, 0, 3)
        .reshape(FC, 128, FF).astype(bf)
    )

    def col_layout(v, n):
        return np.ascontiguousarray(v.reshape(n, 128).T.astype(f32))

    common = {
        "wq": wq_f, "wk": wk_f, "wv": wv_f, "wo": wo_f,
        "w1": w1_f, "w2": w2_f,
        "bq": col_layout(bq_eff, FC), "bk": col_layout(bk_eff, FC),
        "bo": col_layout(bo, FC), "b2": col_layout(b2, FC),
        "b1": col_layout(b1_eff, OC1),
        "bvb": np.ascontiguousarray(
            np.broadcast_to(bv_eff[None, :], (128, H)).astype(f32)
        ),
        "id16": np.eye(128).astype(bf),
        "id32": np.eye(128, dtype=f32),
        "ones": np.ones((1, 128), np.float32),
    }

    in_maps = []
    for c in range(8):
        b, half = divmod(c, 2)
        idx = np.r_[half * QL:(half + 1) * QL, (1 - half) * QL:(2 - half) * QL]
        xp = np.ascontiguousarray(hs[b][idx])
        mb = np.where(am[b][idx] != 0, 0.0, MASK_NEG).astype(np.float32)
        m = dict(common)
        m["x"] = xp
        m["maskb"] = np.ascontiguousarray(mb.reshape(NT, 128).T)
        in_maps.append(m)
    return in_maps


def kernel(**inputs):
    in_maps = _prep_inputs(inputs)
    nc = _get_program(split=True)
    trace = os.environ.get("BASS_KERNEL_TRACE", "") == "1"
    if trace:
        _register_ntff_hook()
    res = run_bass_kernel_spmd(nc, in_maps, list(range(8)), trace=trace)
    LAST["exec_time_ns"] = res.exec_time_ns
    LAST["mean_exec_time_ns"] = res.mean_exec_time_ns
    LAST["res"] = res

    out = np.empty((B, S, H), np.float32)
    for c in range(8):
        b, half = divmod(c, 2)
        out[b, half * QL:(half + 1) * QL] = res.results[c]["out"]
    return out


def _register_ntff_hook():
    """The agent image's antenv lacks axon_hooks; reconstruct it so
    run_bass_kernel_spmd(trace=True) can capture NTFF profiles."""
    import sys, types
    if "antenv.axon_hooks" in sys.modules:
        return
    try:
        import antenv
        from trn_agent_boot.trn_boot import _ntff_profile_via_ctypes
        mod = types.ModuleType("antenv.axon_hooks")
        hook = _ntff_profile_via_ctypes("/opt/axon/libaxon_pjrt.so")
        mod.get_axon_ntff_profile_hook = lambda: hook
        mod.set_axon_ntff_profile_hook = lambda h: None
        sys.modules["antenv.axon_hooks"] = mod
        antenv.axon_hooks = mod
    except Exception:
        pass
